# revision 6
# baseline (speedup 1.0000x reference)
"""Multi-head attention forward on 8 Trainium2 NeuronCores.

Sharding: core c = 2*b + g handles batch b (of 4) and head-group g (8 of 16
heads). Each core computes its group's attention output projected through its
slice of w_proj (row-parallel); the host sums the two partial products per
batch and adds the bias terms.

Math notes (exact identities, not approximations):
  - the key bias b_k adds a per-query constant to every score row, which
    softmax ignores;
  - the value bias b_v passes through attention unchanged (attn rows sum to 1)
    so its projection b_v @ w_proj is folded into the host-side bias;
  - the 1/sqrt(64) score scale is folded into w_q / b_q (exact: power of two).

Compute dtype is float32r (TensorE reduced-precision fp32 path): ~bf16 speed
at N=512 free dim, ~2e-4 matmul error instead of bf16's ~4e-3.
"""

import numpy as np

import concourse.bass as bass
import concourse.tile as tile
from concourse import bacc, mybir
from concourse import bass_utils

F32 = mybir.dt.float32
F32R = mybir.dt.float32r
AF = mybir.ActivationFunctionType

B, S, D = 4, 2048, 1024
H, HD = 16, 64
HG = 8            # heads per core (group)
N_CORES = 8

_CACHE = {}


def _build():
    nc = bacc.Bacc("TRN2", target_bir_lowering=False, debug=False,
                   num_devices=N_CORES)
    # Per-core inputs (distinct data per core, same program).
    xt_d = nc.dram_tensor("xt", [D, S], F32R, kind="ExternalInput").ap()
    wqk_d = nc.dram_tensor("wqk", [D, 2 * HG * HD], F32R, kind="ExternalInput").ap()
    wv_d = nc.dram_tensor("wv", [D, HG * HD], F32R, kind="ExternalInput").ap()
    wp_d = nc.dram_tensor("wp", [HG * HD, D], F32R, kind="ExternalInput").ap()
    bq_d = nc.dram_tensor("bq", [128, 4], F32, kind="ExternalInput").ap()
    out_d = nc.dram_tensor("out", [S, D], F32, kind="ExternalOutput").ap()

    KT = D // 128          # 8 k-tiles over the embedding dim
    ST16 = S // 128        # 16 tiles over sequence
    NCH = S // 512         # 4 sequence chunks of 512

    with tile.TileContext(nc) as tc:
        with (
            tc.tile_pool(name="persist", bufs=1) as pp,
            tc.tile_pool(name="psum", bufs=1, space="PSUM") as ps,
        ):
            # ---- persistent SBUF tensors ----
            qk_sb = [pp.tile([128, S], F32R, name=f"qk{m}", tag=f"qk{m}")
                     for m in range(8)]
            v_sb = [pp.tile([128, HG, HD + 1], F32R, name=f"v{j}", tag=f"v{j}")
                    for j in range(ST16)]
            bq_sb = pp.tile([128, 4], F32, tag="bq")
            nc.sync.dma_start(bq_sb[:], bq_d)
            ones_sb = pp.tile([128, HG, 1], F32, tag="ones")
            nc.vector.memset(ones_sb[:], 1.0)

            def st_psum(nm):
                return ps.tile([128, 1024], F32, tag="st", name=nm, bufs=2)

            # ================= phase A+B: qkT, v (xt streamed by s-half) ====
            with tc.tile_pool(name="xtp", bufs=1) as xtp, \
                 tc.tile_pool(name="wabp", bufs=1) as wabp:
                wqk_sb = [wabp.tile([128, 1024], F32R, name=f"wqk{k}", tag=f"wqk{k}")
                          for k in range(KT)]
                wv_sb = [wabp.tile([128, 512], F32R, name=f"wv{k}", tag=f"wv{k}")
                         for k in range(KT)]
                for k in range(KT):
                    nc.sync.dma_start(wqk_sb[k][:], wqk_d[k * 128:(k + 1) * 128, :])
                    nc.sync.dma_start(wv_sb[k][:], wv_d[k * 128:(k + 1) * 128, :])

                for half in range(2):
                    s0 = half * 1024
                    xt_sb = xtp.tile([128, KT, 1024], F32R, tag="xt")
                    for k in range(KT):
                        nc.sync.dma_start(
                            xt_sb[:, k, :],
                            xt_d[k * 128:(k + 1) * 128, s0:s0 + 1024])
                    # qkT tiles: psum[m, n-chunk] = sum_k wqk[k,m].T @ xt[k,n]
                    for m in range(8):
                        for n in range(2):
                            p = st_psum(f"pa{half}{m}{n}")
                            for k in range(KT):
                                nc.tensor.matmul(
                                    p[:, 0:512],
                                    wqk_sb[k][:, m * 128:(m + 1) * 128],
                                    xt_sb[:, k, n * 512:(n + 1) * 512],
                                    start=(k == 0), stop=(k == KT - 1))
                            dst = qk_sb[m][:, s0 + n * 512:s0 + (n + 1) * 512]
                            if m < 4:  # q tiles: add (pre-scaled) bias on DVE
                                nc.vector.tensor_scalar_add(dst, p[:, 0:512],
                                                            bq_sb[:, m:m + 1])
                            else:      # k tiles: plain copy
                                nc.scalar.activation(dst, p[:, 0:512], AF.Copy,
                                                     bias=0.0, scale=1.0)
                    # v tiles (natural layout): psum[si] = sum_k xt[k,si].T @ wv[k]
                    for si in range(half * 8, half * 8 + 8):
                        p = st_psum(f"pb{si}")
                        for k in range(KT):
                            nc.tensor.matmul(
                                p[:, 0:512],
                                xt_sb[:, k, (si - half * 8) * 128:
                                      (si - half * 8 + 1) * 128],
                                wv_sb[k][:],
                                start=(k == 0), stop=(k == KT - 1))
                        nc.scalar.activation(
                            v_sb[si][:, :, 0:HD],
                            p[:, 0:512].rearrange("p (h d) -> p h d", h=HG),
                            AF.Copy, bias=0.0, scale=1.0)
                        nc.vector.tensor_copy(v_sb[si][:, :, HD:HD + 1],
                                              ones_sb[:])

            # ================= phase C+D: attention per head =================
            with tc.tile_pool(name="attp", bufs=1) as ap, \
                 tc.tile_pool(name="ptp", bufs=3) as ptp, \
                 tc.tile_pool(name="wyp", bufs=1) as wyp, \
                 tc.tile_pool(name="np_", bufs=2) as np_, \
                 tc.tile_pool(name="yp", bufs=2) as yp:
                at_sb = [ap.tile([128, S], F32R, name=f"at{t}", tag=f"at{t}")
                         for t in range(4)]
                wp_sb = [wyp.tile([128, D], F32R, name=f"wp{t}", tag=f"wp{t}")
                        for t in range(4)]
                for t in range(4):
                    nc.sync.dma_start(wp_sb[t][:], wp_d[t * 128:(t + 1) * 128, :])

                for h in range(HG):
                    mt_q, mt_k = h // 2, 4 + h // 2
                    r0 = (h % 2) * 64
                    qT = qk_sb[mt_q][r0:r0 + 64, :]
                    kT = qk_sb[mt_k][r0:r0 + 64, :]
                    for sw in range(2):          # sweep = pair of s_i chunks
                        po = [ps.tile([128, 512], F32, tag="o",
                                      name=f"po{h}{sw}{i}", bufs=4)
                              for i in range(2)]
                        for j in range(ST16):
                            st = st_psum(f"pc{h}{sw}{j}")
                            for il in range(2):
                                ic = sw * 2 + il
                                nc.tensor.matmul(
                                    st[:, il * 512:(il + 1) * 512],
                                    kT[:, j * 128:(j + 1) * 128],
                                    qT[:, ic * 512:(ic + 1) * 512],
                                    start=True, stop=True)
                            pt = ptp.tile([128, 1024], F32R, tag="pt")
                            nc.scalar.activation(pt[:], st[:], AF.Exp,
                                                 bias=0.0, scale=1.0)
                            for il in range(2):
                                nc.tensor.matmul(
                                    po[il][0:HD + 1, :],
                                    v_sb[j][:, h, :],
                                    pt[:, il * 512:(il + 1) * 512],
                                    start=(j == 0), stop=(j == ST16 - 1))
                        # normalize by Z (row 64) and store into attnT (f32r)
                        zrow = np_.tile([1, 1024], F32, tag="zrow")
                        for il in range(2):
                            nc.vector.tensor_copy(zrow[:, il * 512:(il + 1) * 512],
                                                  po[il][64:65, :])
                        inv = np_.tile([1, 1024], F32, tag="inv")
                        nc.vector.reciprocal(inv[:], zrow[:])
                        bc = np_.tile([64, 1024], F32, tag="bc")
                        nc.gpsimd.partition_broadcast(bc[:], inv[:])
                        for il in range(2):
                            ic = sw * 2 + il
                            nc.vector.tensor_mul(
                                at_sb[mt_q][r0:r0 + 64, ic * 512:(ic + 1) * 512],
                                po[il][0:64, :],
                                bc[:, il * 512:(il + 1) * 512])

                # ============= phase E: out = attnT.T @ wp ===================
                for si in range(ST16):
                    for nch in range(2):
                        p = st_psum(f"pe{si}{nch}")
                        for t in range(4):
                            nc.tensor.matmul(
                                p[:, 0:512],
                                at_sb[t][:, si * 128:(si + 1) * 128],
                                wp_sb[t][:, nch * 512:(nch + 1) * 512],
                                start=(t == 0), stop=(t == 3))
                        y = yp.tile([128, 512], F32, tag="y")
                        nc.vector.tensor_copy(y[:], p[:, 0:512])
                        nc.sync.dma_start(
                            out_d[si * 128:(si + 1) * 128,
                                  nch * 512:(nch + 1) * 512], y[:])
    nc.compile()
    return nc


def _prep_inputs(x, w_qkv, b_qkv, w_proj):
    """Host-side shard prep: slice per head-group, fold scale, transpose x."""
    in_maps = []
    for c in range(N_CORES):
        b, g = c // 2, c % 2
        cs = g * 512
        wq = w_qkv[:, cs:cs + 512] * 0.125
        wk = w_qkv[:, 1024 + cs:1024 + cs + 512]
        wv = w_qkv[:, 2048 + cs:2048 + cs + 512]
        bq = (b_qkv[cs:cs + 512] * 0.125).reshape(4, 128).T
        in_maps.append({
            "xt": np.ascontiguousarray(x[b].T),
            "wqk": np.ascontiguousarray(np.concatenate([wq, wk], axis=1)),
            "wv": np.ascontiguousarray(wv),
            "wp": np.ascontiguousarray(w_proj[g * 512:(g + 1) * 512, :]),
            "bq": np.ascontiguousarray(bq.astype(np.float32)),
        })
    return in_maps


def kernel(x, w_qkv, b_qkv, w_proj, b_proj, _trace=False):
    x = np.asarray(x, np.float32)
    w_qkv = np.asarray(w_qkv, np.float32)
    b_qkv = np.asarray(b_qkv, np.float32)
    w_proj = np.asarray(w_proj, np.float32)
    b_proj = np.asarray(b_proj, np.float32)

    if "nc" not in _CACHE:
        _CACHE["nc"] = _build()
    nc = _CACHE["nc"]

    in_maps = _prep_inputs(x, w_qkv, b_qkv, w_proj)
    res = bass_utils.run_bass_kernel_spmd(
        nc, in_maps, core_ids=list(range(N_CORES)), trace=_trace)

    # host-side bias: b_proj plus the value-bias path through w_proj
    bias = b_proj + b_qkv[2048:3072].astype(np.float64) @ w_proj.astype(np.float64)
    bias = bias.astype(np.float32)
    out = np.empty((B, S, D), np.float32)
    for b in range(B):
        out[b] = res.results[2 * b]["out"] + res.results[2 * b + 1]["out"] + bias
    if _trace:
        return out, res
    return out
